# revision 15
# baseline (speedup 1.0000x reference)
"""Bahdanau attention Trainium2 kernel.

Full-input contract: kernel(**inputs) -> (context [64,512] f32, weights [64,2048] f32).
Data-parallel over 8 NeuronCores: 8 batches per core, weights replicated.

Per-core dataflow (all matmuls bf16 in / fp32 PSUM accumulate), two groups of 4
batches pipelined so PE never idles:
  projT[a,s] = sum_e W_enc[e,a] * enc[s,e]     PE: lhsT=W_enc chunk, rhs=encT chunk
  tanhT      = tanh(projT + dec_proj[b] + b)   ACT, per-partition bias
  scores     = w_att . tanhT                   PE: masked-w_att stationary [128,4],
                                               group g lands in PSUM rows 32g..32g+3
  softmax    = additive -1e9 mask, max, exp(+fused accum sum)   DVE/ACT per group
               (normalization by 1/sum happens on host: rsum is an output)
  context    = p @ enc                         PE: lhsT = transposed-p column [128,1],
                                               4 batches concurrent via col-tiling

encoder_outputs is shipped twice in bf16 (natural [S,E] and transposed [E,S]).
Group A's softmax/transpose/context overlap group B's projections; natural-layout
tiles are prefetched so context is never DMA-starved. Consecutive PE matmuls
alternate PSUM banks so fills overlap drains.
"""

import sys

sys.path.insert(0, "/opt/trn_rl_repo")

import numpy as np
import ml_dtypes

import concourse.bass as bass
import concourse.bacc as bacc
import concourse.mybir as mybir
import concourse.tile as tile
from concourse.bass_utils import run_bass_kernel_spmd

BF16 = mybir.dt.bfloat16
F32 = mybir.dt.float32
AF = mybir.ActivationFunctionType
AX = mybir.AxisListType
ALU = mybir.AluOpType

B, S, E, A, D = 64, 2048, 512, 256, 512
NCORES = 8
BL = B // NCORES  # 8 local batches per core
ENC_BUFS = 4
NAT_BUFS = 5

_CACHE = {}

LAST_RESULT = None  # BassKernelResults of most recent run (for test harness)


def _build_nc():
    nc = bacc.Bacc("TRN2", target_bir_lowering=False, debug=False, num_devices=NCORES)

    enc_tr = nc.dram_tensor("enc_tr", [BL, 128, 4 * S], BF16, kind="ExternalInput").ap()
    enc_nat = nc.dram_tensor("enc_nat", [BL, 128, 16 * E], BF16, kind="ExternalInput").ap()
    wenc = nc.dram_tensor("wenc", [128, 4 * A], BF16, kind="ExternalInput").ap()
    wdec = nc.dram_tensor("wdec", [128, 4 * A], BF16, kind="ExternalInput").ap()
    dect = nc.dram_tensor("dect", [128, 4 * BL], BF16, kind="ExternalInput").ap()
    bsum = nc.dram_tensor("bsum", [128, 2], F32, kind="ExternalInput").ap()
    wattm = nc.dram_tensor("wattm", [128, 64], BF16, kind="ExternalInput").ap()
    maskbias = nc.dram_tensor("maskbias", [BL, S], F32, kind="ExternalInput").ap()
    negc = nc.dram_tensor("negc", [36, 1], F32, kind="ExternalInput").ap()
    ident8 = nc.dram_tensor("ident8", [36, 4], F32, kind="ExternalInput").ap()

    ctx_out = nc.dram_tensor("ctx_out", [BL, E], F32, kind="ExternalOutput").ap()
    p_out = nc.dram_tensor("p_out", [36, S], F32, kind="ExternalOutput").ap()
    rsum_out = nc.dram_tensor("rsum_out", [36, 2], F32, kind="ExternalOutput").ap()

    def ap3(t, offset_elems, d1, d2):
        # [d1, 128, d2] dram chunk -> [128 part, d1, d2] view
        return bass.AP(tensor=t.tensor, offset=offset_elems,
                       ap=[[d2, 128], [128 * d2, d1], [1, d2]])

    with tile.TileContext(nc) as tc:
        with (
            tc.tile_pool(name="const", bufs=1) as cpool,
            tc.tile_pool(name="smx", bufs=1) as smx,
            tc.tile_pool(name="encT", bufs=ENC_BUFS) as enc_pool,
            tc.tile_pool(name="nat", bufs=NAT_BUFS) as nat_pool,
            tc.tile_pool(name="tanh", bufs=3) as tanh_pool,
        ):
            enc_tiles = {}

            def load_enc(b):
                encT = enc_pool.tile([128, 4 * S], BF16, tag="encT",
                                     name=f"encT{b}")
                nc.sync.dma_start(encT[:], enc_tr[b])
                enc_tiles[b] = encT

            # big stream first: enc of batch 0 ahead of all constants
            load_enc(0)

            # ---- constants to SBUF (single DMAs) ----
            wenc_sb = cpool.tile([128, 4 * A], BF16)
            nc.sync.dma_start(wenc_sb[:], wenc[:])
            bsum_sb = cpool.tile([128, 2], F32)
            nc.sync.dma_start(bsum_sb[:], bsum[:])

            dpT_sb = cpool.tile([128, 2 * BL], F32)  # dec_proj^T + biases, col h*8+b
            wT0_sb = cpool.tile([128, 64], BF16)  # transposed exp-p group A, col k*4+b
            wT1_sb = cpool.tile([128, 64], BF16)  # group B, col k*4+(b-4)
            p_sb = smx.tile([36, S], F32)
            rsum = smx.tile([36, 2], F32)

            # ---- dec_proj^T [A, BL] = W_dec^T @ dec^T + (b_enc + b_dec) ----
            with (
                tc.tile_pool(name="setup", bufs=1) as spool,
                tc.tile_pool(name="psdp", bufs=2, space="PSUM") as psdp,
            ):
                wdec_sb = spool.tile([128, 4 * A], BF16)
                nc.sync.dma_start(wdec_sb[:], wdec[:])
                dect_sb = spool.tile([128, 4 * BL], BF16)
                nc.sync.dma_start(dect_sb[:], dect[:])
                for h in range(2):
                    ps = psdp.tile([128, BL], F32)
                    for d in range(4):
                        nc.tensor.matmul(
                            ps[:],
                            wdec_sb[:, d * A + h * 128: d * A + h * 128 + 128],
                            dect_sb[:, d * BL:(d + 1) * BL],
                            start=(d == 0),
                            stop=(d == 3),
                        )
                    nc.scalar.activation(
                        dpT_sb[:, h * BL:(h + 1) * BL], ps[:], AF.Identity,
                        bias=bsum_sb[:, h:h + 1], scale=1.0,
                    )

            load_enc(1)
            wattm_sb = cpool.tile([128, 64], BF16)
            nc.sync.dma_start(wattm_sb[:], wattm[:])
            maskb_sb = smx.tile([36, S], F32)
            nc.sync.dma_start(maskb_sb[0:4, :], maskbias[0:4, :])
            nc.sync.dma_start(maskb_sb[32:36, :], maskbias[4:8, :])
            negc_sb = cpool.tile([36, 1], F32)
            nc.sync.dma_start(negc_sb[:], negc[:])
            ident_sb = cpool.tile([36, 4], F32)
            nc.sync.dma_start(ident_sb[:], ident8[:])

            nat_tiles = {}
            with (
                tc.tile_pool(name="psproj", bufs=1, space="PSUM") as psproj,
                tc.tile_pool(name="psscores", bufs=1, space="PSUM") as psscores,
                tc.tile_pool(name="psmisc", bufs=1, space="PSUM") as psmisc,
                tc.tile_pool(name="ctxsb", bufs=4) as ctx_pool,
            ):
                ps_scores = psscores.tile([36, S], F32)

                def load_nat(b):
                    nat = nat_pool.tile([128, 16 * E], BF16, tag="nat",
                                        name=f"nat{b}")
                    nc.sync.dma_start(nat[:], enc_nat[b])
                    nat_tiles[b] = nat

                def proj_batch(b):
                    if b not in enc_tiles:
                        load_enc(b)
                    encT = enc_tiles[b]
                    rowbase = 32 * (b // 4)
                    for h in range(2):
                        tanhT = tanh_pool.tile([128, S], BF16, tag="tanhT",
                                               name=f"tanhT{b}_{h}")
                        for kp in range(2):  # 2 chunks in flight, 3 slots round-robin
                            ks = (2 * kp, 2 * kp + 1)
                            ps_k = [psproj.tile([128, 512], F32,
                                                tag=f"psk{(2 * kp + i) % 3}",
                                                name=f"psk{b}_{h}_{kp}_{i}")
                                    for i in range(2)]
                            for e in range(4):
                                for i, k in enumerate(ks):
                                    nc.tensor.matmul(
                                        ps_k[i][:],
                                        wenc_sb[:, e * A + h * 128: e * A + h * 128 + 128],
                                        encT[:, e * S + k * 512: e * S + k * 512 + 512],
                                        start=(e == 0),
                                        stop=(e == 3),
                                    )
                            for i, k in enumerate(ks):
                                nc.scalar.activation(
                                    tanhT[:, k * 512:(k + 1) * 512], ps_k[i][:],
                                    AF.Tanh,
                                    bias=dpT_sb[:, h * BL + b: h * BL + b + 1],
                                    scale=1.0,
                                )
                            for i, k in enumerate(ks):
                                nc.tensor.matmul(
                                    ps_scores[rowbase:rowbase + 4,
                                              k * 512:(k + 1) * 512],
                                    wattm_sb[:, (b * 2 + h) * 4: (b * 2 + h) * 4 + 4],
                                    tanhT[:, k * 512:(k + 1) * 512],
                                    start=(b % 4 == 0 and h == 0),
                                    stop=(b % 4 == 3 and h == 1),
                                    skip_group_check=True,
                                )

                def softmax_group(g):
                    r = 32 * g
                    nc.vector.tensor_tensor(
                        maskb_sb[r:r + 4, :], ps_scores[r:r + 4, :],
                        maskb_sb[r:r + 4, :], op=ALU.add)
                    for c in range(2):
                        nc.scalar.activation(
                            p_sb[r:r + 4, c * 1024:(c + 1) * 1024],
                            maskb_sb[r:r + 4, c * 1024:(c + 1) * 1024], AF.Exp,
                            bias=negc_sb[r:r + 4, :], scale=1.0,
                            accum_out=rsum[r:r + 4, c:c + 1],
                        )
                    nc.sync.dma_start(p_out[r:r + 4, :], p_sb[r:r + 4, :])
                    nc.sync.dma_start(rsum_out[r:r + 4, :], rsum[r:r + 4, :])

                def transpose_group(g, wT):
                    r = 32 * g
                    ps_t = psmisc.tile([128, 64], F32, tag="psmisc", name=f"pst{g}")
                    for k in range(16):
                        nc.tensor.transpose(
                            ps_t[:, k * 4:(k + 1) * 4],
                            p_sb[r:r + 4, k * 128:(k + 1) * 128],
                            ident_sb[r:r + 4, :],
                        )
                    nc.scalar.activation(wT[:], ps_t[:], AF.Copy, scale=1.0)

                def ctx_group(g, wT):
                    ps_ctx = psmisc.tile([128, E], F32, tag="psmisc",
                                         name=f"psctx{g}")
                    for k in range(16):
                        for i in range(4):
                            b = 4 * g + i
                            nc.tensor.matmul(
                                ps_ctx[32 * i:32 * i + 1, :],
                                wT[:, k * 4 + i: k * 4 + i + 1],
                                nat_tiles[b][:, k * E:(k + 1) * E],
                                start=(k == 0),
                                stop=(k == 15),
                                tile_position=(0, 32 * i),
                                skip_group_check=True,
                            )
                    for i in range(4):
                        b = 4 * g + i
                        ctx_sb = ctx_pool.tile([1, E], F32, tag="ctxsb",
                                               name=f"ctxsb{b}")
                        nc.scalar.activation(
                            ctx_sb[:], ps_ctx[32 * i:32 * i + 1, :],
                            AF.Copy, scale=1.0)
                        nc.sync.dma_start(ctx_out[b:b + 1, :], ctx_sb[:])

                # ---- pipelined schedule ----
                load_enc(2)
                load_enc(3)
                proj_batch(0)
                load_enc(4)
                proj_batch(1)
                load_enc(5)
                proj_batch(2)
                load_enc(6)
                proj_batch(3)
                load_enc(7)
                softmax_group(0)
                proj_batch(4)
                load_nat(0)
                load_nat(1)
                proj_batch(5)
                load_nat(2)
                load_nat(3)
                transpose_group(0, wT0_sb)
                ctx_group(0, wT0_sb)
                proj_batch(6)
                load_nat(4)
                proj_batch(7)
                load_nat(5)
                softmax_group(1)
                warm = psmisc.tile([128, 512], F32, tag="psmisc", name="warm")
                for j in range(14):
                    nc.tensor.matmul(
                        warm[:], wenc_sb[:, 0:128],
                        enc_tiles[7][:, j * 512:(j + 1) * 512],
                        start=(j == 0), stop=(j == 13),
                    )
                load_nat(6)
                load_nat(7)
                transpose_group(1, wT1_sb)
                ctx_group(1, wT1_sb)

    nc.compile()
    return nc


def _get_nc():
    if "nc" not in _CACHE:
        _CACHE["nc"] = _build_nc()
    return _CACHE["nc"]


def kernel(encoder_outputs, decoder_hidden, src_lengths, W_enc, b_enc, W_dec,
           b_dec, w_att, b_att, trace=False):
    global LAST_RESULT
    bf = ml_dtypes.bfloat16
    enc = np.asarray(encoder_outputs, np.float32)
    dec = np.asarray(decoder_hidden, np.float32)
    lens = np.asarray(src_lengths).astype(np.int64)
    W_enc = np.asarray(W_enc, np.float32)
    W_dec = np.asarray(W_dec, np.float32)
    b_enc = np.asarray(b_enc, np.float32)
    b_dec = np.asarray(b_dec, np.float32)
    w_att = np.asarray(w_att, np.float32)

    enc_bf = enc.astype(bf)
    # partition-major layouts: dram row p holds exactly SBUF partition p's bytes
    enc_nat = np.ascontiguousarray(
        enc_bf.reshape(B, 16, 128, E).transpose(0, 2, 1, 3)).reshape(B, 128, 16 * E)
    enc_tr = np.ascontiguousarray(enc_bf.transpose(0, 2, 1)
                                  .reshape(B, 4, 128, S).transpose(0, 2, 1, 3)
                                  ).reshape(B, 128, 4 * S)

    wenc_h = np.ascontiguousarray(
        W_enc.astype(bf).reshape(4, 128, A).transpose(1, 0, 2)).reshape(128, 4 * A)
    wdec_h = np.ascontiguousarray(
        W_dec.astype(bf).reshape(4, 128, A).transpose(1, 0, 2)).reshape(128, 4 * A)
    bsum_h = np.ascontiguousarray(
        (b_enc + b_dec).astype(np.float32).reshape(2, 128).T)
    wattm_h = np.zeros((128, 64), np.float32)
    for b in range(BL):
        for h in range(2):
            wattm_h[:, (b * 2 + h) * 4 + (b % 4)] = w_att[h * 128:(h + 1) * 128]
    wattm_h = wattm_h.astype(bf)
    ident_h = np.zeros((36, 4), np.float32)
    ident_h[0:4] = np.eye(4)
    ident_h[32:36] = np.eye(4)
    score_bound = float(np.abs(w_att).sum()) + 1.0
    negc_h = np.full((36, 1), -score_bound, np.float32)
    s_iota = np.arange(S)

    in_maps = []
    for c in range(NCORES):
        sl = slice(c * BL, (c + 1) * BL)
        mb = np.where(s_iota[None, :] < lens[sl, None], 0.0, -1e9).astype(np.float32)
        dect_h = np.ascontiguousarray(
            dec[sl].T.reshape(4, 128, BL).transpose(1, 0, 2)).astype(bf).reshape(128, 4 * BL)
        in_maps.append({
            "enc_tr": np.ascontiguousarray(enc_tr[sl]),
            "enc_nat": np.ascontiguousarray(enc_nat[sl]),
            "wenc": wenc_h, "wdec": wdec_h, "dect": dect_h, "bsum": bsum_h,
            "wattm": wattm_h, "maskbias": mb, "ident8": ident_h, "negc": negc_h,
        })

    nc = _get_nc()
    res = run_bass_kernel_spmd(nc, in_maps, list(range(NCORES)), trace=trace)
    LAST_RESULT = res

    rows = np.r_[0:4, 32:36]
    ctx = np.concatenate([np.asarray(r["ctx_out"]) for r in res.results], axis=0)
    p = np.concatenate([np.asarray(r["p_out"])[rows] for r in res.results], axis=0)
    rsum = np.concatenate(
        [np.asarray(r["rsum_out"])[rows].sum(axis=1, keepdims=True)
         for r in res.results], axis=0)
    zero = lens == 0
    rsum[zero] = S  # all-masked rows: exp underflows to 0; reference is uniform
    context = ctx / rsum
    weights = p / rsum
    if zero.any():
        weights[zero] = 1.0 / S
        context[zero] = enc[zero].mean(axis=1)
    return context.astype(np.float32), weights.astype(np.float32)


if __name__ == "__main__":
    rng = np.random.default_rng(0)
    ins = {
        "encoder_outputs": rng.standard_normal((B, S, E), np.float32),
        "decoder_hidden": rng.standard_normal((B, D), np.float32),
        "src_lengths": rng.integers(0, S, (B,)),
        "W_enc": rng.standard_normal((E, A), np.float32) / np.sqrt(E),
        "b_enc": rng.standard_normal((A,), np.float32) * 0.01,
        "W_dec": rng.standard_normal((D, A), np.float32) / np.sqrt(D),
        "b_dec": rng.standard_normal((A,), np.float32) * 0.01,
        "w_att": rng.standard_normal((A,), np.float32) / np.sqrt(A),
        "b_att": np.float32(0.01),
    }
    c, w = kernel(**ins)
    print("ctx", c.shape, "w", w.shape)


# revision 16
# speedup vs baseline: 1.0233x; 1.0233x over previous
"""Bahdanau attention Trainium2 kernel.

Full-input contract: kernel(**inputs) -> (context [64,512] f32, weights [64,2048] f32).
Data-parallel over 8 NeuronCores: 8 batches per core, weights replicated.

Per-core dataflow (all matmuls bf16 in / fp32 PSUM accumulate), two groups of 4
batches pipelined so PE never idles:
  projT[a,s] = sum_e W_enc[e,a] * enc[s,e]     PE: lhsT=W_enc chunk, rhs=encT chunk
  tanhT      = tanh(projT + dec_proj[b] + b)   ACT, per-partition bias
  scores     = w_att . tanhT                   PE: masked-w_att stationary [128,4],
                                               group g lands in PSUM rows 32g..32g+3
  softmax    = additive -1e9 mask, max, exp(+fused accum sum)   DVE/ACT per group
               (normalization by 1/sum happens on host: rsum is an output)
  context    = p @ enc                         PE: lhsT = transposed-p column [128,1],
                                               4 batches concurrent via col-tiling

encoder_outputs is shipped twice in bf16 (natural [S,E] and transposed [E,S]).
Group A's softmax/transpose/context overlap group B's projections; natural-layout
tiles are prefetched so context is never DMA-starved. Consecutive PE matmuls
alternate PSUM banks so fills overlap drains.
"""

import sys

sys.path.insert(0, "/opt/trn_rl_repo")

import numpy as np
import ml_dtypes

import concourse.bass as bass
import concourse.bacc as bacc
import concourse.mybir as mybir
import concourse.tile as tile
from concourse.bass_utils import run_bass_kernel_spmd

BF16 = mybir.dt.bfloat16
F32 = mybir.dt.float32
AF = mybir.ActivationFunctionType
AX = mybir.AxisListType
ALU = mybir.AluOpType

B, S, E, A, D = 64, 2048, 512, 256, 512
NCORES = 8
BL = B // NCORES  # 8 local batches per core
ENC_BUFS = 4
NAT_BUFS = 5

_CACHE = {}

LAST_RESULT = None  # BassKernelResults of most recent run (for test harness)


def _build_nc():
    nc = bacc.Bacc("TRN2", target_bir_lowering=False, debug=False, num_devices=NCORES)

    enc_tr = nc.dram_tensor("enc_tr", [BL, 128, 4 * S], BF16, kind="ExternalInput").ap()
    enc_nat = nc.dram_tensor("enc_nat", [BL, 128, 16 * E], BF16, kind="ExternalInput").ap()
    wenc = nc.dram_tensor("wenc", [128, 4 * A], BF16, kind="ExternalInput").ap()
    wdec = nc.dram_tensor("wdec", [128, 4 * A], BF16, kind="ExternalInput").ap()
    dect = nc.dram_tensor("dect", [128, 4 * BL], BF16, kind="ExternalInput").ap()
    bsum = nc.dram_tensor("bsum", [128, 2], F32, kind="ExternalInput").ap()
    wattm = nc.dram_tensor("wattm", [128, 64], BF16, kind="ExternalInput").ap()
    maskbias = nc.dram_tensor("maskbias", [BL, S], F32, kind="ExternalInput").ap()
    negc = nc.dram_tensor("negc", [36, 1], F32, kind="ExternalInput").ap()
    ident8 = nc.dram_tensor("ident8", [36, 4], F32, kind="ExternalInput").ap()

    ctx_out = nc.dram_tensor("ctx_out", [BL, E], F32, kind="ExternalOutput").ap()
    p_out = nc.dram_tensor("p_out", [36, S], F32, kind="ExternalOutput").ap()
    rsum_out = nc.dram_tensor("rsum_out", [36, 2], F32, kind="ExternalOutput").ap()

    def ap3(t, offset_elems, d1, d2):
        # [d1, 128, d2] dram chunk -> [128 part, d1, d2] view
        return bass.AP(tensor=t.tensor, offset=offset_elems,
                       ap=[[d2, 128], [128 * d2, d1], [1, d2]])

    with tile.TileContext(nc) as tc:
        with (
            tc.tile_pool(name="const", bufs=1) as cpool,
            tc.tile_pool(name="smx", bufs=1) as smx,
            tc.tile_pool(name="encT", bufs=ENC_BUFS) as enc_pool,
            tc.tile_pool(name="nat", bufs=NAT_BUFS) as nat_pool,
            tc.tile_pool(name="tanh", bufs=3) as tanh_pool,
        ):
            enc_tiles = {}

            def load_enc(b):
                encT = enc_pool.tile([128, 4 * S], BF16, tag="encT",
                                     name=f"encT{b}")
                nc.sync.dma_start(encT[:], enc_tr[b])
                enc_tiles[b] = encT

            # big stream first: enc of batch 0 ahead of all constants
            load_enc(0)

            # ---- constants to SBUF (single DMAs) ----
            wenc_sb = cpool.tile([128, 4 * A], BF16)
            nc.sync.dma_start(wenc_sb[:], wenc[:])
            bsum_sb = cpool.tile([128, 2], F32)
            nc.sync.dma_start(bsum_sb[:], bsum[:])

            dpT_sb = cpool.tile([128, 2 * BL], F32)  # dec_proj^T + biases, col h*8+b
            wT0_sb = cpool.tile([128, 64], BF16)  # transposed exp-p group A, col k*4+b
            wT1_sb = cpool.tile([128, 64], BF16)  # group B, col k*4+(b-4)
            p_sb = smx.tile([36, S], F32)
            rsum = smx.tile([36, 2], F32)

            # ---- dec_proj^T [A, BL] = W_dec^T @ dec^T + (b_enc + b_dec) ----
            with (
                tc.tile_pool(name="setup", bufs=1) as spool,
                tc.tile_pool(name="psdp", bufs=2, space="PSUM") as psdp,
            ):
                wdec_sb = spool.tile([128, 4 * A], BF16)
                nc.sync.dma_start(wdec_sb[:], wdec[:])
                dect_sb = spool.tile([128, 4 * BL], BF16)
                nc.sync.dma_start(dect_sb[:], dect[:])
                for h in range(2):
                    ps = psdp.tile([128, BL], F32)
                    for d in range(4):
                        nc.tensor.matmul(
                            ps[:],
                            wdec_sb[:, d * A + h * 128: d * A + h * 128 + 128],
                            dect_sb[:, d * BL:(d + 1) * BL],
                            start=(d == 0),
                            stop=(d == 3),
                        )
                    nc.scalar.activation(
                        dpT_sb[:, h * BL:(h + 1) * BL], ps[:], AF.Identity,
                        bias=bsum_sb[:, h:h + 1], scale=1.0,
                    )

            load_enc(1)
            wattm_sb = cpool.tile([128, 64], BF16)
            nc.sync.dma_start(wattm_sb[:], wattm[:])
            maskb_sb = smx.tile([36, S], F32)
            nc.sync.dma_start(maskb_sb[0:4, :], maskbias[0:4, :])
            nc.sync.dma_start(maskb_sb[32:36, :], maskbias[4:8, :])
            negc_sb = cpool.tile([36, 1], F32)
            nc.sync.dma_start(negc_sb[:], negc[:])
            ident_sb = cpool.tile([36, 4], F32)
            nc.sync.dma_start(ident_sb[:], ident8[:])

            nat_tiles = {}
            with (
                tc.tile_pool(name="psproj", bufs=1, space="PSUM") as psproj,
                tc.tile_pool(name="psscores", bufs=1, space="PSUM") as psscores,
                tc.tile_pool(name="psmisc", bufs=1, space="PSUM") as psmisc,
                tc.tile_pool(name="ctxsb", bufs=4) as ctx_pool,
            ):
                ps_scores = psscores.tile([36, S], F32)

                def load_nat(b):
                    nat = nat_pool.tile([128, 16 * E], BF16, tag="nat",
                                        name=f"nat{b}")
                    nc.sync.dma_start(nat[:], enc_nat[b])
                    nat_tiles[b] = nat

                def proj_batch(b):
                    if b not in enc_tiles:
                        load_enc(b)
                    encT = enc_tiles[b]
                    rowbase = 32 * (b // 4)
                    for h in range(2):
                        tanhT = tanh_pool.tile([128, S], BF16, tag="tanhT",
                                               name=f"tanhT{b}_{h}")
                        for kp in range(2):  # 2 chunks in flight, 3 slots round-robin
                            ks = (2 * kp, 2 * kp + 1)
                            ps_k = [psproj.tile([128, 512], F32,
                                                tag=f"psk{(2 * kp + i) % 3}",
                                                name=f"psk{b}_{h}_{kp}_{i}")
                                    for i in range(2)]
                            for e in range(4):
                                for i, k in enumerate(ks):
                                    nc.tensor.matmul(
                                        ps_k[i][:],
                                        wenc_sb[:, e * A + h * 128: e * A + h * 128 + 128],
                                        encT[:, e * S + k * 512: e * S + k * 512 + 512],
                                        start=(e == 0),
                                        stop=(e == 3),
                                    )
                            for i, k in enumerate(ks):
                                nc.scalar.activation(
                                    tanhT[:, k * 512:(k + 1) * 512], ps_k[i][:],
                                    AF.Tanh,
                                    bias=dpT_sb[:, h * BL + b: h * BL + b + 1],
                                    scale=1.0,
                                )
                            for i, k in enumerate(ks):
                                nc.tensor.matmul(
                                    ps_scores[rowbase:rowbase + 4,
                                              k * 512:(k + 1) * 512],
                                    wattm_sb[:, (b * 2 + h) * 4: (b * 2 + h) * 4 + 4],
                                    tanhT[:, k * 512:(k + 1) * 512],
                                    start=(b % 4 == 0 and h == 0),
                                    stop=(b % 4 == 3 and h == 1),
                                    skip_group_check=True,
                                )

                def softmax_group(g):
                    r = 32 * g
                    nc.vector.tensor_tensor(
                        maskb_sb[r:r + 4, :], ps_scores[r:r + 4, :],
                        maskb_sb[r:r + 4, :], op=ALU.add)
                    for c in range(2):
                        nc.scalar.activation(
                            p_sb[r:r + 4, c * 1024:(c + 1) * 1024],
                            maskb_sb[r:r + 4, c * 1024:(c + 1) * 1024], AF.Exp,
                            bias=negc_sb[r:r + 4, :], scale=1.0,
                            accum_out=rsum[r:r + 4, c:c + 1],
                        )
                    nc.sync.dma_start(p_out[r:r + 4, :], p_sb[r:r + 4, :])
                    nc.sync.dma_start(rsum_out[r:r + 4, :], rsum[r:r + 4, :])

                def transpose_group(g, wT):
                    r = 32 * g
                    ps_t = psmisc.tile([128, 64], F32, tag="psmisc", name=f"pst{g}")
                    for k in range(16):
                        nc.tensor.transpose(
                            ps_t[:, k * 4:(k + 1) * 4],
                            p_sb[r:r + 4, k * 128:(k + 1) * 128],
                            ident_sb[r:r + 4, :],
                        )
                    nc.scalar.activation(wT[:], ps_t[:], AF.Copy, scale=1.0)

                def ctx_group(g, wT):
                    ps_ctx = psmisc.tile([128, E], F32, tag="psmisc",
                                         name=f"psctx{g}")
                    for k in range(16):
                        for i in range(4):
                            b = 4 * g + i
                            nc.tensor.matmul(
                                ps_ctx[32 * i:32 * i + 1, :],
                                wT[:, k * 4 + i: k * 4 + i + 1],
                                nat_tiles[b][:, k * E:(k + 1) * E],
                                start=(k == 0),
                                stop=(k == 15),
                                tile_position=(0, 32 * i),
                                skip_group_check=True,
                            )
                    for i in range(4):
                        b = 4 * g + i
                        ctx_sb = ctx_pool.tile([1, E], F32, tag="ctxsb",
                                               name=f"ctxsb{b}")
                        nc.scalar.activation(
                            ctx_sb[:], ps_ctx[32 * i:32 * i + 1, :],
                            AF.Copy, scale=1.0)
                        nc.sync.dma_start(ctx_out[b:b + 1, :], ctx_sb[:])

                # ---- pipelined schedule ----
                for b in range(4):
                    proj_batch(b)
                softmax_group(0)
                proj_batch(4)
                load_nat(0)
                load_nat(1)
                proj_batch(5)
                load_nat(2)
                load_nat(3)
                transpose_group(0, wT0_sb)
                ctx_group(0, wT0_sb)
                proj_batch(6)
                load_nat(4)
                proj_batch(7)
                load_nat(5)
                softmax_group(1)
                warm = psmisc.tile([128, 512], F32, tag="psmisc", name="warm")
                for j in range(14):
                    nc.tensor.matmul(
                        warm[:], wenc_sb[:, 0:128],
                        enc_tiles[7][:, j * 512:(j + 1) * 512],
                        start=(j == 0), stop=(j == 13),
                    )
                load_nat(6)
                load_nat(7)
                transpose_group(1, wT1_sb)
                ctx_group(1, wT1_sb)

    nc.compile()
    return nc


def _get_nc():
    if "nc" not in _CACHE:
        _CACHE["nc"] = _build_nc()
    return _CACHE["nc"]


def kernel(encoder_outputs, decoder_hidden, src_lengths, W_enc, b_enc, W_dec,
           b_dec, w_att, b_att, trace=False):
    global LAST_RESULT
    bf = ml_dtypes.bfloat16
    enc = np.asarray(encoder_outputs, np.float32)
    dec = np.asarray(decoder_hidden, np.float32)
    lens = np.asarray(src_lengths).astype(np.int64)
    W_enc = np.asarray(W_enc, np.float32)
    W_dec = np.asarray(W_dec, np.float32)
    b_enc = np.asarray(b_enc, np.float32)
    b_dec = np.asarray(b_dec, np.float32)
    w_att = np.asarray(w_att, np.float32)

    enc_bf = enc.astype(bf)
    # partition-major layouts: dram row p holds exactly SBUF partition p's bytes
    enc_nat = np.ascontiguousarray(
        enc_bf.reshape(B, 16, 128, E).transpose(0, 2, 1, 3)).reshape(B, 128, 16 * E)
    enc_tr = np.ascontiguousarray(enc_bf.transpose(0, 2, 1)
                                  .reshape(B, 4, 128, S).transpose(0, 2, 1, 3)
                                  ).reshape(B, 128, 4 * S)

    wenc_h = np.ascontiguousarray(
        W_enc.astype(bf).reshape(4, 128, A).transpose(1, 0, 2)).reshape(128, 4 * A)
    wdec_h = np.ascontiguousarray(
        W_dec.astype(bf).reshape(4, 128, A).transpose(1, 0, 2)).reshape(128, 4 * A)
    bsum_h = np.ascontiguousarray(
        (b_enc + b_dec).astype(np.float32).reshape(2, 128).T)
    wattm_h = np.zeros((128, 64), np.float32)
    for b in range(BL):
        for h in range(2):
            wattm_h[:, (b * 2 + h) * 4 + (b % 4)] = w_att[h * 128:(h + 1) * 128]
    wattm_h = wattm_h.astype(bf)
    ident_h = np.zeros((36, 4), np.float32)
    ident_h[0:4] = np.eye(4)
    ident_h[32:36] = np.eye(4)
    score_bound = float(np.abs(w_att).sum()) + 1.0
    negc_h = np.full((36, 1), -score_bound, np.float32)
    s_iota = np.arange(S)

    in_maps = []
    for c in range(NCORES):
        sl = slice(c * BL, (c + 1) * BL)
        mb = np.where(s_iota[None, :] < lens[sl, None], 0.0, -1e9).astype(np.float32)
        dect_h = np.ascontiguousarray(
            dec[sl].T.reshape(4, 128, BL).transpose(1, 0, 2)).astype(bf).reshape(128, 4 * BL)
        in_maps.append({
            "enc_tr": np.ascontiguousarray(enc_tr[sl]),
            "enc_nat": np.ascontiguousarray(enc_nat[sl]),
            "wenc": wenc_h, "wdec": wdec_h, "dect": dect_h, "bsum": bsum_h,
            "wattm": wattm_h, "maskbias": mb, "ident8": ident_h, "negc": negc_h,
        })

    nc = _get_nc()
    res = run_bass_kernel_spmd(nc, in_maps, list(range(NCORES)), trace=trace)
    LAST_RESULT = res

    rows = np.r_[0:4, 32:36]
    ctx = np.concatenate([np.asarray(r["ctx_out"]) for r in res.results], axis=0)
    p = np.concatenate([np.asarray(r["p_out"])[rows] for r in res.results], axis=0)
    rsum = np.concatenate(
        [np.asarray(r["rsum_out"])[rows].sum(axis=1, keepdims=True)
         for r in res.results], axis=0)
    zero = lens == 0
    rsum[zero] = S  # all-masked rows: exp underflows to 0; reference is uniform
    context = ctx / rsum
    weights = p / rsum
    if zero.any():
        weights[zero] = 1.0 / S
        context[zero] = enc[zero].mean(axis=1)
    return context.astype(np.float32), weights.astype(np.float32)


if __name__ == "__main__":
    rng = np.random.default_rng(0)
    ins = {
        "encoder_outputs": rng.standard_normal((B, S, E), np.float32),
        "decoder_hidden": rng.standard_normal((B, D), np.float32),
        "src_lengths": rng.integers(0, S, (B,)),
        "W_enc": rng.standard_normal((E, A), np.float32) / np.sqrt(E),
        "b_enc": rng.standard_normal((A,), np.float32) * 0.01,
        "W_dec": rng.standard_normal((D, A), np.float32) / np.sqrt(D),
        "b_dec": rng.standard_normal((A,), np.float32) * 0.01,
        "w_att": rng.standard_normal((A,), np.float32) / np.sqrt(A),
        "b_att": np.float32(0.01),
    }
    c, w = kernel(**ins)
    print("ctx", c.shape, "w", w.shape)


# revision 17
# speedup vs baseline: 1.0457x; 1.0219x over previous
"""Bahdanau attention Trainium2 kernel.

Full-input contract: kernel(**inputs) -> (context [64,512] f32, weights [64,2048] f32).
Data-parallel over 8 NeuronCores: 8 batches per core, weights replicated.

Per-core dataflow (all matmuls bf16 in / fp32 PSUM accumulate), two groups of 4
batches pipelined so PE never idles:
  projT[a,s] = sum_e W_enc[e,a] * enc[s,e]     PE: lhsT=W_enc chunk, rhs=encT chunk
  tanhT      = tanh(projT + dec_proj[b] + b)   ACT, per-partition bias
  scores     = w_att . tanhT                   PE: masked-w_att stationary [128,4],
                                               group g lands in PSUM rows 32g..32g+3
  softmax    = additive -1e9 mask, max, exp(+fused accum sum)   DVE/ACT per group
               (normalization by 1/sum happens on host: rsum is an output)
  context    = p @ enc                         PE: lhsT = transposed-p column [128,1],
                                               4 batches concurrent via col-tiling

encoder_outputs is shipped twice in bf16 (natural [S,E] and transposed [E,S]).
Group A's softmax/transpose/context overlap group B's projections; natural-layout
tiles are prefetched so context is never DMA-starved. Consecutive PE matmuls
alternate PSUM banks so fills overlap drains.
"""

import sys

sys.path.insert(0, "/opt/trn_rl_repo")

import numpy as np
import ml_dtypes

import concourse.bass as bass
import concourse.bacc as bacc
import concourse.mybir as mybir
import concourse.tile as tile
from concourse.bass_utils import run_bass_kernel_spmd

BF16 = mybir.dt.bfloat16
F32 = mybir.dt.float32
AF = mybir.ActivationFunctionType
AX = mybir.AxisListType
ALU = mybir.AluOpType

B, S, E, A, D = 64, 2048, 512, 256, 512
NCORES = 8
BL = B // NCORES  # 8 local batches per core
ENC_BUFS = 4
NAT_BUFS = 5

_CACHE = {}

LAST_RESULT = None  # BassKernelResults of most recent run (for test harness)


def _build_nc():
    nc = bacc.Bacc("TRN2", target_bir_lowering=False, debug=False, num_devices=NCORES)

    enc_tr = nc.dram_tensor("enc_tr", [BL, 128, 4 * S], BF16, kind="ExternalInput").ap()
    enc_nat = nc.dram_tensor("enc_nat", [BL, 128, 16 * E], BF16, kind="ExternalInput").ap()
    wenc = nc.dram_tensor("wenc", [128, 4 * A], BF16, kind="ExternalInput").ap()
    wdec = nc.dram_tensor("wdec", [128, 4 * A], BF16, kind="ExternalInput").ap()
    dect = nc.dram_tensor("dect", [128, 4 * BL], BF16, kind="ExternalInput").ap()
    bsum = nc.dram_tensor("bsum", [128, 2], F32, kind="ExternalInput").ap()
    wattm = nc.dram_tensor("wattm", [128, 64], BF16, kind="ExternalInput").ap()
    maskbias = nc.dram_tensor("maskbias", [BL, S], F32, kind="ExternalInput").ap()
    negc = nc.dram_tensor("negc", [36, 1], F32, kind="ExternalInput").ap()
    ident8 = nc.dram_tensor("ident8", [36, 4], F32, kind="ExternalInput").ap()

    ctx_out = nc.dram_tensor("ctx_out", [BL, E], F32, kind="ExternalOutput").ap()
    p_out = nc.dram_tensor("p_out", [36, S], F32, kind="ExternalOutput").ap()
    rsum_out = nc.dram_tensor("rsum_out", [36, 2], F32, kind="ExternalOutput").ap()

    def ap3(t, offset_elems, d1, d2):
        # [d1, 128, d2] dram chunk -> [128 part, d1, d2] view
        return bass.AP(tensor=t.tensor, offset=offset_elems,
                       ap=[[d2, 128], [128 * d2, d1], [1, d2]])

    with tile.TileContext(nc) as tc:
        with (
            tc.tile_pool(name="const", bufs=1) as cpool,
            tc.tile_pool(name="smx", bufs=1) as smx,
            tc.tile_pool(name="encT", bufs=ENC_BUFS) as enc_pool,
            tc.tile_pool(name="nat", bufs=NAT_BUFS) as nat_pool,
            tc.tile_pool(name="tanh", bufs=2) as tanh_pool,
        ):
            enc_tiles = {}

            def load_enc(b):
                encT = enc_pool.tile([128, 4 * S], BF16, tag="encT",
                                     name=f"encT{b}")
                nc.sync.dma_start(encT[:], enc_tr[b])
                enc_tiles[b] = encT

            # big stream first: enc of batch 0 ahead of all constants
            load_enc(0)

            # ---- constants to SBUF (single DMAs) ----
            wenc_sb = cpool.tile([128, 4 * A], BF16)
            nc.sync.dma_start(wenc_sb[:], wenc[:])
            bsum_sb = cpool.tile([128, 2], F32)
            nc.sync.dma_start(bsum_sb[:], bsum[:])

            dpT_sb = cpool.tile([128, 2 * BL], F32)  # dec_proj^T + biases, col h*8+b
            wT0_sb = cpool.tile([128, 64], BF16)  # transposed exp-p group A, col k*4+b
            wT1_sb = cpool.tile([128, 64], BF16)  # group B, col k*4+(b-4)
            p_sb = smx.tile([36, S], F32)
            rsum = smx.tile([36, 2], F32)

            # ---- dec_proj^T [A, BL] = W_dec^T @ dec^T + (b_enc + b_dec) ----
            with (
                tc.tile_pool(name="setup", bufs=1) as spool,
                tc.tile_pool(name="psdp", bufs=2, space="PSUM") as psdp,
            ):
                wdec_sb = spool.tile([128, 4 * A], BF16)
                nc.sync.dma_start(wdec_sb[:], wdec[:])
                dect_sb = spool.tile([128, 4 * BL], BF16)
                nc.sync.dma_start(dect_sb[:], dect[:])
                for h in range(2):
                    ps = psdp.tile([128, BL], F32)
                    for d in range(4):
                        nc.tensor.matmul(
                            ps[:],
                            wdec_sb[:, d * A + h * 128: d * A + h * 128 + 128],
                            dect_sb[:, d * BL:(d + 1) * BL],
                            start=(d == 0),
                            stop=(d == 3),
                        )
                    nc.scalar.activation(
                        dpT_sb[:, h * BL:(h + 1) * BL], ps[:], AF.Identity,
                        bias=bsum_sb[:, h:h + 1], scale=1.0,
                    )

            load_enc(1)
            wattm_sb = cpool.tile([128, 64], BF16)
            nc.sync.dma_start(wattm_sb[:], wattm[:])
            maskb_sb = smx.tile([36, S], F32)
            nc.sync.dma_start(maskb_sb[0:4, :], maskbias[0:4, :])
            nc.sync.dma_start(maskb_sb[32:36, :], maskbias[4:8, :])
            negc_sb = cpool.tile([36, 1], F32)
            nc.sync.dma_start(negc_sb[:], negc[:])
            ident_sb = cpool.tile([36, 4], F32)
            nc.sync.dma_start(ident_sb[:], ident8[:])

            nat_tiles = {}
            with (
                tc.tile_pool(name="psproj", bufs=1, space="PSUM") as psproj,
                tc.tile_pool(name="psscores", bufs=1, space="PSUM") as psscores,
                tc.tile_pool(name="psmisc", bufs=1, space="PSUM") as psmisc,
                tc.tile_pool(name="ctxsb", bufs=4) as ctx_pool,
            ):
                ps_scores = psscores.tile([36, S], F32)

                def load_nat(b):
                    nat = nat_pool.tile([128, 16 * E], BF16, tag="nat",
                                        name=f"nat{b}")
                    nc.sync.dma_start(nat[:], enc_nat[b])
                    nat_tiles[b] = nat

                def proj_batch(b):
                    if b not in enc_tiles:
                        load_enc(b)
                    encT = enc_tiles[b]
                    rowbase = 32 * (b // 4)
                    for h in range(2):
                        tanhT = tanh_pool.tile([128, S], BF16, tag="tanhT",
                                               name=f"tanhT{b}_{h}")
                        for kp in range(2):  # 2 chunks in flight, 3 slots round-robin
                            ks = (2 * kp, 2 * kp + 1)
                            ps_k = [psproj.tile([128, 512], F32,
                                                tag=f"psk{(2 * kp + i) % 3}",
                                                name=f"psk{b}_{h}_{kp}_{i}")
                                    for i in range(2)]
                            for e in range(4):
                                for i, k in enumerate(ks):
                                    nc.tensor.matmul(
                                        ps_k[i][:],
                                        wenc_sb[:, e * A + h * 128: e * A + h * 128 + 128],
                                        encT[:, e * S + k * 512: e * S + k * 512 + 512],
                                        start=(e == 0),
                                        stop=(e == 3),
                                    )
                            for i, k in enumerate(ks):
                                nc.scalar.activation(
                                    tanhT[:, k * 512:(k + 1) * 512], ps_k[i][:],
                                    AF.Tanh,
                                    bias=dpT_sb[:, h * BL + b: h * BL + b + 1],
                                    scale=1.0,
                                )
                            for i, k in enumerate(ks):
                                nc.tensor.matmul(
                                    ps_scores[rowbase:rowbase + 4,
                                              k * 512:(k + 1) * 512],
                                    wattm_sb[:, (b * 2 + h) * 4: (b * 2 + h) * 4 + 4],
                                    tanhT[:, k * 512:(k + 1) * 512],
                                    start=(b % 4 == 0 and h == 0),
                                    stop=(b % 4 == 3 and h == 1),
                                    skip_group_check=True,
                                )

                def softmax_group(g):
                    r = 32 * g
                    nc.vector.tensor_tensor(
                        maskb_sb[r:r + 4, :], ps_scores[r:r + 4, :],
                        maskb_sb[r:r + 4, :], op=ALU.add)
                    for c in range(2):
                        nc.scalar.activation(
                            p_sb[r:r + 4, c * 1024:(c + 1) * 1024],
                            maskb_sb[r:r + 4, c * 1024:(c + 1) * 1024], AF.Exp,
                            bias=negc_sb[r:r + 4, :], scale=1.0,
                            accum_out=rsum[r:r + 4, c:c + 1],
                        )
                    nc.sync.dma_start(p_out[r:r + 4, :], p_sb[r:r + 4, :])
                    nc.sync.dma_start(rsum_out[r:r + 4, :], rsum[r:r + 4, :])

                def transpose_group(g, wT):
                    r = 32 * g
                    ps_t = psmisc.tile([128, 64], F32, tag="psmisc", name=f"pst{g}")
                    for k in range(16):
                        nc.tensor.transpose(
                            ps_t[:, k * 4:(k + 1) * 4],
                            p_sb[r:r + 4, k * 128:(k + 1) * 128],
                            ident_sb[r:r + 4, :],
                        )
                    nc.scalar.activation(wT[:], ps_t[:], AF.Copy, scale=1.0)

                def ctx_group(g, wT):
                    ps_ctx = psmisc.tile([128, E], F32, tag="psmisc",
                                         name=f"psctx{g}")
                    for k in range(16):
                        for i in range(4):
                            b = 4 * g + i
                            nc.tensor.matmul(
                                ps_ctx[32 * i:32 * i + 1, :],
                                wT[:, k * 4 + i: k * 4 + i + 1],
                                nat_tiles[b][:, k * E:(k + 1) * E],
                                start=(k == 0),
                                stop=(k == 15),
                                tile_position=(0, 32 * i),
                                skip_group_check=True,
                            )
                    for i in range(4):
                        b = 4 * g + i
                        ctx_sb = ctx_pool.tile([1, E], F32, tag="ctxsb",
                                               name=f"ctxsb{b}")
                        nc.scalar.activation(
                            ctx_sb[:], ps_ctx[32 * i:32 * i + 1, :],
                            AF.Copy, scale=1.0)
                        nc.sync.dma_start(ctx_out[b:b + 1, :], ctx_sb[:])

                # ---- pipelined schedule ----
                for b in range(4):
                    proj_batch(b)
                softmax_group(0)
                proj_batch(4)
                load_nat(0)
                load_nat(1)
                proj_batch(5)
                load_nat(2)
                load_nat(3)
                transpose_group(0, wT0_sb)
                ctx_group(0, wT0_sb)
                proj_batch(6)
                load_nat(4)
                proj_batch(7)
                load_nat(5)
                softmax_group(1)
                warm = psmisc.tile([128, 512], F32, tag="psmisc", name="warm")
                for j in range(14):
                    nc.tensor.matmul(
                        warm[:], wenc_sb[:, 0:128],
                        enc_tiles[7][:, j * 512:(j + 1) * 512],
                        start=(j == 0), stop=(j == 13),
                    )
                load_nat(6)
                load_nat(7)
                transpose_group(1, wT1_sb)
                ctx_group(1, wT1_sb)

    nc.compile()
    return nc


def _get_nc():
    if "nc" not in _CACHE:
        _CACHE["nc"] = _build_nc()
    return _CACHE["nc"]


def kernel(encoder_outputs, decoder_hidden, src_lengths, W_enc, b_enc, W_dec,
           b_dec, w_att, b_att, trace=False):
    global LAST_RESULT
    bf = ml_dtypes.bfloat16
    enc = np.asarray(encoder_outputs, np.float32)
    dec = np.asarray(decoder_hidden, np.float32)
    lens = np.asarray(src_lengths).astype(np.int64)
    W_enc = np.asarray(W_enc, np.float32)
    W_dec = np.asarray(W_dec, np.float32)
    b_enc = np.asarray(b_enc, np.float32)
    b_dec = np.asarray(b_dec, np.float32)
    w_att = np.asarray(w_att, np.float32)

    enc_bf = enc.astype(bf)
    # partition-major layouts: dram row p holds exactly SBUF partition p's bytes
    enc_nat = np.ascontiguousarray(
        enc_bf.reshape(B, 16, 128, E).transpose(0, 2, 1, 3)).reshape(B, 128, 16 * E)
    enc_tr = np.ascontiguousarray(enc_bf.transpose(0, 2, 1)
                                  .reshape(B, 4, 128, S).transpose(0, 2, 1, 3)
                                  ).reshape(B, 128, 4 * S)

    wenc_h = np.ascontiguousarray(
        W_enc.astype(bf).reshape(4, 128, A).transpose(1, 0, 2)).reshape(128, 4 * A)
    wdec_h = np.ascontiguousarray(
        W_dec.astype(bf).reshape(4, 128, A).transpose(1, 0, 2)).reshape(128, 4 * A)
    bsum_h = np.ascontiguousarray(
        (b_enc + b_dec).astype(np.float32).reshape(2, 128).T)
    wattm_h = np.zeros((128, 64), np.float32)
    for b in range(BL):
        for h in range(2):
            wattm_h[:, (b * 2 + h) * 4 + (b % 4)] = w_att[h * 128:(h + 1) * 128]
    wattm_h = wattm_h.astype(bf)
    ident_h = np.zeros((36, 4), np.float32)
    ident_h[0:4] = np.eye(4)
    ident_h[32:36] = np.eye(4)
    score_bound = float(np.abs(w_att).sum()) + 1.0
    negc_h = np.full((36, 1), -score_bound, np.float32)
    s_iota = np.arange(S)

    in_maps = []
    for c in range(NCORES):
        sl = slice(c * BL, (c + 1) * BL)
        mb = np.where(s_iota[None, :] < lens[sl, None], 0.0, -1e9).astype(np.float32)
        dect_h = np.ascontiguousarray(
            dec[sl].T.reshape(4, 128, BL).transpose(1, 0, 2)).astype(bf).reshape(128, 4 * BL)
        in_maps.append({
            "enc_tr": np.ascontiguousarray(enc_tr[sl]),
            "enc_nat": np.ascontiguousarray(enc_nat[sl]),
            "wenc": wenc_h, "wdec": wdec_h, "dect": dect_h, "bsum": bsum_h,
            "wattm": wattm_h, "maskbias": mb, "ident8": ident_h, "negc": negc_h,
        })

    nc = _get_nc()
    res = run_bass_kernel_spmd(nc, in_maps, list(range(NCORES)), trace=trace)
    LAST_RESULT = res

    rows = np.r_[0:4, 32:36]
    ctx = np.concatenate([np.asarray(r["ctx_out"]) for r in res.results], axis=0)
    p = np.concatenate([np.asarray(r["p_out"])[rows] for r in res.results], axis=0)
    rsum = np.concatenate(
        [np.asarray(r["rsum_out"])[rows].sum(axis=1, keepdims=True)
         for r in res.results], axis=0)
    zero = lens == 0
    rsum[zero] = S  # all-masked rows: exp underflows to 0; reference is uniform
    context = ctx / rsum
    weights = p / rsum
    if zero.any():
        weights[zero] = 1.0 / S
        context[zero] = enc[zero].mean(axis=1)
    return context.astype(np.float32), weights.astype(np.float32)


if __name__ == "__main__":
    rng = np.random.default_rng(0)
    ins = {
        "encoder_outputs": rng.standard_normal((B, S, E), np.float32),
        "decoder_hidden": rng.standard_normal((B, D), np.float32),
        "src_lengths": rng.integers(0, S, (B,)),
        "W_enc": rng.standard_normal((E, A), np.float32) / np.sqrt(E),
        "b_enc": rng.standard_normal((A,), np.float32) * 0.01,
        "W_dec": rng.standard_normal((D, A), np.float32) / np.sqrt(D),
        "b_dec": rng.standard_normal((A,), np.float32) * 0.01,
        "w_att": rng.standard_normal((A,), np.float32) / np.sqrt(A),
        "b_att": np.float32(0.01),
    }
    c, w = kernel(**ins)
    print("ctx", c.shape, "w", w.shape)
